# revision 1
# baseline (speedup 1.0000x reference)
"""Trainium2 kernel for nn_Baseline_LatentSet_Single (vq_codebook).

Strategy (per sharding hint): pure data parallel. The batch dimension
n=64 is sharded 8-ways across the 8 NeuronCores; latent_basis (tiny
codebook) is replicated. Each core runs the full per-sample pipeline
(3x3 avg-pool support features, energy gating, cosine routing to the
per-stripe codebook, token pooling, top-k presence, and the scatter of
per-stripe maps back to full resolution). No cross-device communication
is needed; outputs are gathered (concatenated along n) on the host.

The per-shard program is compiled for the NeuronCores through the
device's native compiler (axon PJRT -> neuronx), one replica per core,
launched with pmap across all 8 cores.
"""

import numpy as np
import jax
import jax.numpy as jnp

# ---- problem constants (hardcoded per contract; kernel.py is self-contained)
G = 16          # horizontal stripes
K = 4           # latents per stripe
TEMP = 0.125
TOPK_RATIO = 0.125
GATE_THR = 0.05
EPS = 1e-6
KSZ = 3

N_FULL, C, H, W = 64, 256, 64, 44
N_CORES = 8
N_LOC = N_FULL // N_CORES


def _avg_pool3(x):
    # x: [n, g, c, sh, w]; 3x3 avg pool, stride 1, pad 1, count_include_pad=True
    s = jax.lax.reduce_window(x, 0.0, jax.lax.add,
                              (1, 1, 1, KSZ, KSZ), (1, 1, 1, 1, 1),
                              [(0, 0), (0, 0), (0, 0), (1, 1), (1, 1)])
    return s / float(KSZ * KSZ)


def _l2norm(v, axis=-1):
    return v / jnp.maximum(jnp.linalg.norm(v, axis=axis, keepdims=True), 1e-12)


def _shard_fn(x, latent_basis):
    """Per-core computation on a local batch shard x: [n_loc, c, h, w]."""
    n, c, h, w = x.shape
    sh = h // G
    p = sh * w
    xr = x.reshape(n, c, G, sh, w).transpose(0, 2, 1, 3, 4)
    support_feat = _avg_pool3(xr)
    feat_flat = xr.transpose(0, 1, 3, 4, 2).reshape(n, G, p, c)
    support_flat = support_feat.transpose(0, 1, 3, 4, 2).reshape(n, G, p, c)

    # active mask from detached feature energy
    e = jax.lax.stop_gradient(xr).astype(jnp.float32)
    fe = jnp.mean(e * e, axis=2)
    fe = fe / jnp.maximum(jnp.max(fe, axis=(-2, -1), keepdims=True), EPS)
    active = (fe > GATE_THR).astype(jnp.float32)
    fallback = (fe > 0).astype(jnp.float32)
    use_fb = jnp.sum(active, axis=(-2, -1), keepdims=True) <= 0
    active = jnp.where(use_fb, fallback, active).reshape(n, G, p)

    # cosine-similarity routing
    sfn = _l2norm(support_flat)
    lbn = _l2norm(latent_basis)
    logits = jnp.einsum('ngpc,gkc->ngpk', sfn, lbn) / TEMP
    route_prob = jax.nn.softmax(logits, axis=-1)
    support = route_prob * active[..., None]
    pool_w = support / jnp.maximum(jnp.sum(support, axis=2, keepdims=True), EPS)
    tokens = jnp.einsum('ngpk,ngpc->ngkc', pool_w, feat_flat)
    tokens = tokens.transpose(0, 3, 1, 2).reshape(n, c, G * K)

    # top-k presence (normalized per-sample -> stays shard-local)
    kk = max(1, min(p, int(p * TOPK_RATIO)))
    topv, _ = jax.lax.top_k(support.transpose(0, 1, 3, 2), kk)
    presence = jnp.mean(topv, axis=-1).reshape(n, G * K)
    presence = presence / jnp.maximum(jnp.sum(presence, axis=1, keepdims=True), EPS)

    # scatter per-stripe maps back into full-resolution zero tensors
    eye = jnp.eye(G, dtype=support.dtype)
    loc_s = support.reshape(n, G, sh, w, K).transpose(0, 1, 4, 2, 3)
    loc_p = pool_w.reshape(n, G, sh, w, K).transpose(0, 1, 4, 2, 3)
    smap = jnp.einsum('ngksw,gG->ngkGsw', loc_s, eye).reshape(n, G * K, h, w)
    pmap_ = jnp.einsum('ngksw,gG->ngkGsw', loc_p, eye).reshape(n, G * K, h, w)
    support_flat_global = smap.transpose(0, 2, 3, 1).reshape(n, h * w, G * K)
    pool_weights_flat_global = pmap_.transpose(0, 2, 3, 1).reshape(n, h * w, G * K)

    return (tokens.astype(x.dtype), presence.astype(x.dtype),
            support_flat_global.astype(x.dtype),
            pool_weights_flat_global.astype(x.dtype))


_COMPILED = None


def _get_compiled():
    global _COMPILED
    if _COMPILED is None:
        devs = jax.devices()[:N_CORES]
        _COMPILED = jax.pmap(_shard_fn, axis_name='i', devices=devs)
    return _COMPILED


def kernel(x, latent_basis):
    x = np.asarray(x, dtype=np.float32)
    latent_basis = np.asarray(latent_basis, dtype=np.float32)
    # shard batch across the 8 cores; replicate the codebook
    xs = x.reshape(N_CORES, N_LOC, C, H, W)
    lbs = np.broadcast_to(latent_basis, (N_CORES,) + latent_basis.shape)
    fn = _get_compiled()
    tokens, presence, smap, pmap_ = fn(xs, lbs)
    # gather: concatenate the batch shards back together on host
    tokens = np.asarray(tokens).reshape(N_FULL, C, G * K)
    presence = np.asarray(presence).reshape(N_FULL, G * K)
    smap = np.asarray(smap).reshape(N_FULL, H * W, G * K)
    pmap_ = np.asarray(pmap_).reshape(N_FULL, H * W, G * K)
    return (tokens, presence, smap, pmap_)


# revision 2
# speedup vs baseline: 21.0849x; 21.0849x over previous
"""Trainium2 kernel for nn_Baseline_LatentSet_Single (vq_codebook).

Strategy (per sharding hint): pure data parallel. The batch dimension
n=64 is sharded 8-ways across the 8 NeuronCores; latent_basis (tiny
codebook) is replicated. Each core runs the full per-sample pipeline
(3x3 avg-pool support features, energy gating, cosine routing to the
per-stripe codebook, token pooling, top-k presence, and the scatter of
per-stripe maps back to full resolution). No cross-device communication
is needed; outputs are gathered (concatenated along n) on the host.

The per-shard program is compiled for the NeuronCores through the
device's native compiler (axon PJRT -> neuronx), one replica per core,
launched with pmap across all 8 cores.
"""

import numpy as np
import jax
import jax.numpy as jnp

# ---- problem constants (hardcoded per contract; kernel.py is self-contained)
G = 16          # horizontal stripes
K = 4           # latents per stripe
TEMP = 0.125
TOPK_RATIO = 0.125
GATE_THR = 0.05
EPS = 1e-6
KSZ = 3

N_FULL, C, H, W = 64, 256, 64, 44
N_CORES = 8
N_LOC = N_FULL // N_CORES


def _avg_pool3(x):
    # x: [n, g, c, sh, w]; 3x3 avg pool, stride 1, pad 1, count_include_pad=True
    s = jax.lax.reduce_window(x, 0.0, jax.lax.add,
                              (1, 1, 1, KSZ, KSZ), (1, 1, 1, 1, 1),
                              [(0, 0), (0, 0), (0, 0), (1, 1), (1, 1)])
    return s / float(KSZ * KSZ)


def _l2norm(v, axis=-1):
    return v / jnp.maximum(jnp.linalg.norm(v, axis=axis, keepdims=True), 1e-12)


def _shard_fn(x, latent_basis):
    """Per-core computation on a local batch shard x: [n_loc, c, h, w]."""
    n, c, h, w = x.shape
    sh = h // G
    p = sh * w
    xr = x.reshape(n, c, G, sh, w).transpose(0, 2, 1, 3, 4)
    support_feat = _avg_pool3(xr)
    feat_flat = xr.transpose(0, 1, 3, 4, 2).reshape(n, G, p, c)
    support_flat = support_feat.transpose(0, 1, 3, 4, 2).reshape(n, G, p, c)

    # active mask from detached feature energy
    e = jax.lax.stop_gradient(xr).astype(jnp.float32)
    fe = jnp.mean(e * e, axis=2)
    fe = fe / jnp.maximum(jnp.max(fe, axis=(-2, -1), keepdims=True), EPS)
    active = (fe > GATE_THR).astype(jnp.float32)
    fallback = (fe > 0).astype(jnp.float32)
    use_fb = jnp.sum(active, axis=(-2, -1), keepdims=True) <= 0
    active = jnp.where(use_fb, fallback, active).reshape(n, G, p)

    # cosine-similarity routing
    sfn = _l2norm(support_flat)
    lbn = _l2norm(latent_basis)
    logits = jnp.einsum('ngpc,gkc->ngpk', sfn, lbn) / TEMP
    route_prob = jax.nn.softmax(logits, axis=-1)
    support = route_prob * active[..., None]
    pool_w = support / jnp.maximum(jnp.sum(support, axis=2, keepdims=True), EPS)
    tokens = jnp.einsum('ngpk,ngpc->ngkc', pool_w, feat_flat)
    tokens = tokens.transpose(0, 3, 1, 2).reshape(n, c, G * K)

    # top-k presence (normalized per-sample -> stays shard-local)
    kk = max(1, min(p, int(p * TOPK_RATIO)))
    topv, _ = jax.lax.top_k(support.transpose(0, 1, 3, 2), kk)
    presence = jnp.mean(topv, axis=-1).reshape(n, G * K)
    presence = presence / jnp.maximum(jnp.sum(presence, axis=1, keepdims=True), EPS)

    # scatter per-stripe maps back into full-resolution zero tensors.
    # Row index of the flat map is g*p + (s*w + wi), i.e. exactly the (G, p)
    # flattening of `support`, so the maps are block-diagonal expansions with
    # no layout transposes needed: out[n, (g,q), (g',k)] = delta(g,g')*val.
    eye = jnp.eye(G, dtype=support.dtype)
    support_flat_global = (
        support[:, :, :, None, :] * eye[None, :, None, :, None]
    ).reshape(n, h * w, G * K)
    pool_weights_flat_global = (
        pool_w[:, :, :, None, :] * eye[None, :, None, :, None]
    ).reshape(n, h * w, G * K)

    return (tokens.astype(x.dtype), presence.astype(x.dtype),
            support_flat_global.astype(x.dtype),
            pool_weights_flat_global.astype(x.dtype))


_COMPILED = None


def _get_compiled():
    global _COMPILED
    if _COMPILED is None:
        devs = jax.devices()[:N_CORES]
        _COMPILED = jax.pmap(_shard_fn, axis_name='i', devices=devs)
    return _COMPILED


def kernel(x, latent_basis):
    x = np.asarray(x, dtype=np.float32)
    latent_basis = np.asarray(latent_basis, dtype=np.float32)
    # shard batch across the 8 cores; replicate the codebook
    xs = x.reshape(N_CORES, N_LOC, C, H, W)
    lbs = np.broadcast_to(latent_basis, (N_CORES,) + latent_basis.shape)
    fn = _get_compiled()
    tokens, presence, smap, pmap_ = fn(xs, lbs)
    # gather: concatenate the batch shards back together on host
    tokens = np.asarray(tokens).reshape(N_FULL, C, G * K)
    presence = np.asarray(presence).reshape(N_FULL, G * K)
    smap = np.asarray(smap).reshape(N_FULL, H * W, G * K)
    pmap_ = np.asarray(pmap_).reshape(N_FULL, H * W, G * K)
    return (tokens, presence, smap, pmap_)
